# revision 1
# baseline (speedup 1.0000x reference)
"""Trainium2 Bass kernel for nn_CurvedMultiHeadAttention (B=4, S=1024, E=768, H=12, D=64, R=16).

Sharding: 8 cores; core c handles batch b=c//2 and heads h0=6*(c%2) .. h0+5
(head-parallel within a batch element). Each core computes a partial output
(its 6 heads' out-projection contribution, with bo/2 folded in); the host sums
the two partials per batch element (the unshard step for head sharding).

Math restructuring (validated against the reference at ~1e-6 rel err):
 - softmax over keys is invariant to per-query additive shifts => the qq term
   of the Mahalanobis distance drops entirely.
 - the EPS*I part of G_h contributes <1e-5 relative to scores => dropped.
 - scoresT[t,s] = sum_r kAT[r,t]*qAT[r,s];  per-key bias ckk[t] = -SCALE*kk[t]
   + mask[t] is applied as the ScalarE activation bias during exp (scoresT has
   keys on partitions, so the bias axis is the partition axis).
 - qA = (hidden @ Wq^T + bq) @ A is computed as hidden @ (Wq^T A) + bq A:
   Weff = A^T Wq is built on-device with tiny matmuls, so the big projection
   emits 16 (padded to 32) output dims per head instead of 64.
 - softmax denominator comes free as a ones column appended to v in the ctx
   matmul; bo/2 is added via a GpSimd-broadcast row during PSUM evacuation.

All heavy matmuls are bf16 with fp32 PSUM accumulation (measured end-to-end
rel err ~2.4e-3). The score/exp stage of head h+1 is software-pipelined with
the ctx stage of head h so ScalarE (exp) and TensorE overlap.
"""

import os
import numpy as np
import ml_dtypes

import concourse.bass as bass
import concourse.tile as tile
from concourse import bacc
from concourse import mybir
from concourse.bass_utils import run_bass_kernel_spmd
from concourse.masks import make_identity

F32 = mybir.dt.float32
BF16 = mybir.dt.bfloat16
AF = mybir.ActivationFunctionType

S = 1024          # sequence length
E = 768           # embed
D = 64            # head dim
R = 16            # rank
HPC = 6           # heads per core
NCORES = 8
SCALE = 1.0 / 8.0
ESC = 2.0 * SCALE  # exp scale

EAUG = E + 1            # 769 (ones row folds biases into the projections)
KCH = [128] * 6 + [1]   # contraction chunking of EAUG
WEFFW = 32 * HPC        # Weff columns, 32 per head (16 real + 16 pad)

LAST_RESULTS = None     # BassKernelResults of the most recent run (for test.py)


def _emit(tc):
    nc = tc.nc
    hTa = nc.dram_tensor("hTa", [EAUG, S], BF16, kind="ExternalInput")
    wqr = nc.dram_tensor("Wqr", [HPC * D, EAUG], BF16, kind="ExternalInput")
    wkr = nc.dram_tensor("Wkr", [HPC * D, EAUG], BF16, kind="ExternalInput")
    wvd = nc.dram_tensor("WvTa", [EAUG, HPC * D], BF16, kind="ExternalInput")
    wod = nc.dram_tensor("WoT", [HPC * D, E], BF16, kind="ExternalInput")
    apd = nc.dram_tensor("Apack", [D, WEFFW], BF16, kind="ExternalInput")
    mkd = nc.dram_tensor("maskT", [128, S // 128], F32, kind="ExternalInput")
    bod = nc.dram_tensor("bo2", [1, E], F32, kind="ExternalInput")
    outd = nc.dram_tensor("outp", [S, E], F32, kind="ExternalOutput")

    import contextlib
    stack = contextlib.ExitStack()
    const = stack.enter_context(tc.tile_pool(name="const", bufs=1))
    work = stack.enter_context(tc.tile_pool(name="work", bufs=4))
    ptp = stack.enter_context(tc.tile_pool(name="ptp", bufs=16))
    psp = stack.enter_context(tc.tile_pool(name="psp", bufs=3, space="PSUM"))

    def psum():
        return psp.tile([128, 1024], F32, name="ps", tag="ps")

    def psum_bf():
        return psp.tile([128, 1024], BF16, name="pst", tag="pst", bufs=2)

    dma = nc.sync.dma_start
    cp = nc.vector.tensor_copy

    # ---------------- constant / weight loads ----------------
    hT, wv = [], []
    for i, kc in enumerate(KCH):
        r0 = 128 * i
        hT.append(const.tile([kc, S], BF16, name=f"hT{i}", tag=f"hT{i}"))
        dma(out=hT[i][:, :], in_=hTa[r0:r0 + kc, :])
        wv.append(const.tile([kc, HPC * D], BF16, name=f"wv{i}", tag=f"wv{i}"))
        dma(out=wv[i][:, :], in_=wvd[r0:r0 + kc, :])
    wqh, wkh = [], []
    for h in range(HPC):
        wqh.append(const.tile([D, EAUG], BF16, name=f"wqh{h}", tag=f"wqh{h}"))
        dma(out=wqh[h][:, :], in_=wqr[D * h:D * (h + 1), :])
        wkh.append(const.tile([D, EAUG], BF16, name=f"wkh{h}", tag=f"wkh{h}"))
        dma(out=wkh[h][:, :], in_=wkr[D * h:D * (h + 1), :])
    wo = []
    for i in range(3):
        wo.append(const.tile([128, E], BF16, name=f"wo{i}", tag=f"wo{i}"))
        dma(out=wo[i][:, :], in_=wod[128 * i:128 * (i + 1), :])
    apk = const.tile([D, WEFFW], BF16, name="apk", tag="apk")
    dma(out=apk[:, :], in_=apd[:, :])
    maskT = const.tile([128, S // 128], F32, name="maskT", tag="maskT")
    dma(out=maskT[:, :], in_=mkd[:, :])
    bo2 = const.tile([1, E], F32, name="bo2", tag="bo2")
    dma(out=bo2[:, :], in_=bod[:, :])
    bo_bc = const.tile([128, E], F32, name="bo_bc", tag="bo_bc")
    nc.gpsimd.partition_broadcast(bo_bc[:, :], bo2[:, :])

    ones16 = const.tile([R, 1], BF16, name="ones16", tag="ones16")
    nc.vector.memset(ones16[:, :], 1.0)
    ident = const.tile([128, 128], BF16, name="ident", tag="ident")
    make_identity(nc, ident[:, :])

    vsb = [const.tile([128, HPC * (D + 1)], BF16, name=f"v{t}", tag=f"v{t}") for t in range(8)]
    ctxn = [const.tile([128, HPC * D], BF16, name=f"ctxn{s}", tag=f"ctxn{s}") for s in range(8)]
    ctxT = [const.tile([128, S], BF16, name=f"ctxT{j}", tag=f"ctxT{j}") for j in range(3)]

    # ---------------- v projection -> vsb (bf16, ones col interleaved) --------
    for t in range(8):
        pv = psum()
        for k in range(7):
            nc.tensor.matmul(
                out=pv[:, 0:HPC * D],
                lhsT=hT[k][:, 128 * t:128 * (t + 1)],
                rhs=wv[k][:, :],
                start=(k == 0), stop=(k == 6),
            )
        vst = work.tile([128, HPC * D], BF16, name="vst", tag="vst", bufs=2)
        cp(vst[:, :], pv[:, 0:HPC * D])
        vv = vsb[t][:, :].rearrange("p (h c) -> p h c", h=HPC)   # (128, 6, 65)
        cp(vv[:, :, 0:D], vst[:, :].rearrange("p (h c) -> p h c", h=HPC))
        nc.vector.memset(vv[:, :, D:D + 1], 1.0)

    # ---------------- Weff = [A^T Wq ; A^T bq] on device ----------------
    # weff{q,k}[ec] : (128|1, 192) bf16, rows = e (769 total), col 32h+r
    weff = {}
    for key, wh in (("q", wqh), ("k", wkh)):
        tiles = []
        for ec, kc in enumerate(KCH):
            pw = psum()
            for h in range(HPC):
                nc.tensor.matmul(
                    out=pw[0:kc, 32 * h:32 * h + R],
                    lhsT=wh[h][:, 128 * ec:128 * ec + kc],
                    rhs=apk[:, 32 * h:32 * h + R],
                    start=True, stop=True,
                )
            wt = const.tile([kc, WEFFW], BF16, name=f"weff{key}{ec}",
                            tag=f"weff{key}{ec}")
            cp(wt[:, :], pw[0:kc, 0:WEFFW])
            tiles.append(wt)
        weff[key] = tiles

    # ---------------- qAT/kAT for all heads: Weff.T @ hTa ----------------
    # two partition groups: heads 0-3 (cols 0:128) and heads 4-5 (cols 128:192)
    qk = {"q": [], "k": []}
    for key in ("q", "k"):
        for mt, mp in ((0, 128), (1, 64)):
            pq = psum()
            for n in range(2):
                for k in range(7):
                    nc.tensor.matmul(
                        out=pq[0:mp, 512 * n:512 * (n + 1)],
                        lhsT=weff[key][k][:, 128 * mt:128 * mt + mp],
                        rhs=hT[k][:, 512 * n:512 * (n + 1)],
                        start=(k == 0), stop=(k == 6),
                    )
            big = work.tile([128, S], BF16, name=f"{key}all{mt}", tag=f"{key}all", bufs=2)
            cp(big[0:mp, :], pq[0:mp, :])
            # per-head base-0 slices via fast bf16->bf16 copies
            for hh in range(4 if mt == 0 else 2):
                th = work.tile([R, S], BF16, name=f"{key}a", tag=f"{key}a", bufs=7)
                cp(th[:, :], big[32 * hh:32 * hh + R, :])
                qk[key].append(th)

    # ---------------- per-head attention, software-pipelined ----------------
    def stage_a(h):
        """scores + exp for head h; returns the 8 PT tiles."""
        qa, ka = qk["q"][h], qk["k"][h]
        ksq = work.tile([R, S], BF16, name="ksq", tag="ksq", bufs=2)
        nc.vector.tensor_mul(ksq[:, :], ka[:, :], ka[:, :])
        pk = psum()
        for t in range(8):
            nc.tensor.matmul(
                out=pk[:, t:t + 1],
                lhsT=ksq[:, 128 * t:128 * (t + 1)],
                rhs=ones16[:, :],
                start=True, stop=True,
            )
        ckkT = work.tile([128, S // 128], F32, name="ckkT", tag="ckkT", bufs=2)
        nc.vector.scalar_tensor_tensor(
            out=ckkT[:, :], in0=pk[:, 0:S // 128], scalar=-SCALE,
            in1=maskT[:, :], op0=mybir.AluOpType.mult, op1=mybir.AluOpType.add,
        )
        pts = []
        for t in range(8):
            pc = psum()
            for n in range(2):
                nc.tensor.matmul(
                    out=pc[:, 512 * n:512 * (n + 1)],
                    lhsT=ka[:, 128 * t:128 * (t + 1)],
                    rhs=qa[:, 512 * n:512 * (n + 1)],
                    start=True, stop=True,
                )
            pt_t = ptp.tile([128, S], BF16, name="pt", tag="pt")
            nc.scalar.activation(out=pt_t[:, :], in_=pc[:, :],
                                 func=AF.Exp, bias=ckkT[:, t:t + 1], scale=ESC)
            pts.append(pt_t)
        return pts

    def stage_b(h, pts):
        """ctx + normalize for head h."""
        for s in range(8):
            px = psum()
            for t in range(8):
                nc.tensor.matmul(
                    out=px[:, 0:D + 1],
                    lhsT=pts[t][:, 128 * s:128 * (s + 1)],
                    rhs=vsb[t][:, (D + 1) * h:(D + 1) * (h + 1)],
                    start=(t == 0), stop=(t == 7),
                )
            rec = work.tile([128, 1], F32, name="rec", tag="rec")
            nc.vector.reciprocal(rec[:, :], px[:, D:D + 1])
            nc.vector.tensor_scalar_mul(ctxn[s][:, D * h:D * (h + 1)],
                                        px[:, 0:D], rec[:, 0:1])

    prev = stage_a(0)
    for h in range(HPC):
        nxt = stage_a(h + 1) if h + 1 < HPC else None
        stage_b(h, prev)
        prev = nxt

    # ---------------- transpose ctxn -> ctxT (384, S) ----------------
    for s in range(8):
        for j in range(3):
            pt_ps = psum_bf()
            nc.tensor.transpose(pt_ps[:, 0:128], ctxn[s][:, 128 * j:128 * (j + 1)],
                                ident[:, :])
            cp(ctxT[j][:, 128 * s:128 * (s + 1)], pt_ps[:, 0:128])

    # ---------------- out projection + bo/2 + store -------------
    for s in range(8):
        po = psum()
        for n0, nw in ((0, 512), (512, 256)):
            for kc in range(3):
                nc.tensor.matmul(
                    out=po[:, n0:n0 + nw],
                    lhsT=ctxT[kc][:, 128 * s:128 * (s + 1)],
                    rhs=wo[kc][:, n0:n0 + nw],
                    start=(kc == 0), stop=(kc == 2),
                )
        osb = work.tile([128, E], F32, name="osb", tag="osb", bufs=2)
        nc.vector.scalar_tensor_tensor(
            out=osb[:, :], in0=po[:, 0:E], scalar=0.0,
            in1=bo_bc[:, :], op0=mybir.AluOpType.bypass, op1=mybir.AluOpType.add,
        )
        dma(out=outd[128 * s:128 * (s + 1), :], in_=osb[:, :])

    stack.close()


_NC_CACHE = None


def _build():
    global _NC_CACHE
    if _NC_CACHE is None:
        nc = bacc.Bacc("TRN2", target_bir_lowering=False, debug=False,
                       enable_asserts=True, num_devices=NCORES)
        with tile.TileContext(nc) as tc:
            _emit(tc)
        nc.compile()
        _NC_CACHE = nc
    return _NC_CACHE


def kernel(hidden_states, attention_mask, Wq, bq, Wk, bk, Wv, bv, Wo, bo, A,
           **_ignored):
    global LAST_RESULTS
    hidden_states = np.asarray(hidden_states, np.float32)
    attention_mask = np.asarray(attention_mask, np.float32)
    Wq, bq = np.asarray(Wq, np.float32), np.asarray(bq, np.float32)
    Wk, bk = np.asarray(Wk, np.float32), np.asarray(bk, np.float32)
    Wv, bv = np.asarray(Wv, np.float32), np.asarray(bv, np.float32)
    Wo, bo = np.asarray(Wo, np.float32), np.asarray(bo, np.float32)
    A = np.asarray(A, np.float32)

    B = hidden_states.shape[0]
    nc = _build()

    bf = ml_dtypes.bfloat16
    ones1 = np.ones((1, S), np.float32)
    in_maps = []
    for c in range(NCORES):
        b = c // 2
        h0 = HPC * (c % 2)
        sl = slice(h0 * D, (h0 + HPC) * D)
        hTa = np.concatenate([hidden_states[b].T, ones1], 0)
        Wqr = np.concatenate([Wq[sl], bq[sl][:, None]], 1)        # (384, 769)
        Wkr = np.concatenate([Wk[sl], bk[sl][:, None]], 1)
        WvTa = np.concatenate([Wv[sl].T, bv[sl][None, :]], 0)     # (769, 384)
        WoT = Wo[:, sl].T.copy()                                  # (384, 768)
        Apack = np.zeros((D, WEFFW), np.float32)
        for h in range(HPC):
            Apack[:, 32 * h:32 * h + R] = A[h0 + h]
        maskT = attention_mask[b, 0, 0].reshape(S // 128, 128).T
        in_maps.append({
            "hTa": np.ascontiguousarray(hTa.astype(bf)),
            "Wqr": np.ascontiguousarray(Wqr.astype(bf)),
            "Wkr": np.ascontiguousarray(Wkr.astype(bf)),
            "WvTa": np.ascontiguousarray(WvTa.astype(bf)),
            "WoT": np.ascontiguousarray(WoT.astype(bf)),
            "Apack": np.ascontiguousarray(Apack.astype(bf)),
            "maskT": np.ascontiguousarray(maskT),
            "bo2": np.ascontiguousarray((bo / 2.0)[None, :]),
        })

    res = run_bass_kernel_spmd(nc, in_maps, list(range(NCORES)),
                               trace=bool(os.environ.get("KERNEL_TRACE")))
    LAST_RESULTS = res
    parts = [res.results[c]["outp"] for c in range(NCORES)]
    out = np.stack([parts[2 * b] + parts[2 * b + 1] for b in range(B)], 0)
    return np.ascontiguousarray(out.astype(np.float32))



# revision 10
# speedup vs baseline: 1.0882x; 1.0882x over previous
"""Trainium2 Bass kernel for nn_CurvedMultiHeadAttention (B=4, S=1024, E=768, H=12, D=64, R=16).

Sharding: 8 cores; core c handles batch b=c//2 and heads h0=6*(c%2) .. h0+5.
Each core computes its 6 heads' out-projection contribution (bo/2 folded in);
the host sums the two partials per batch element.

Math (validated ~1e-6 vs reference in fp64):
 - softmax over keys is invariant to per-query shifts => qq term drops.
 - EPS*I part of G_h contributes <1e-5 to scores => dropped.
 - Weff_q = A_h^T Wq_h (16x768 per head) is folded on the HOST (weight-only
   transform), so the device projects hidden straight to qA/kA (16 dims/head,
   padded to 32 for PE quadrant alignment).
 - q/k biases fold into the per-key exp bias:
     exp arg = ESC*(kA0.qA0) + ckk[t],
     ckk = -SCALE*|kA0|^2 + kA0.(ESC*(bqA-bkA)) + mask
   computed with two small indicator matmuls (ksq x ind + kA x bvec).
 - softmax denominator comes free as a ones column appended to v in the ctx
   matmul; normalization happens during PSUM evacuation (per-partition scalar).

Schedule: weights-first multi-queue DMA; qk projection (PE) streams behind the
hT chunk DMAs; exp (ScalarE) starts ~16us in and runs back-to-back; v
projection, ctx, transposes and out-projection j-chunks fill PE slack under
the exp stream; 2-wave out-projection tail.
"""

import os
import numpy as np
import ml_dtypes

import concourse.bass as bass
import concourse.tile as tile
from concourse import bacc
from concourse import mybir
from concourse.bass_utils import run_bass_kernel_spmd

F32 = mybir.dt.float32
BF16 = mybir.dt.bfloat16
AF = mybir.ActivationFunctionType
ALU = mybir.AluOpType

S = 1024          # sequence length
E = 768           # embed
D = 64            # head dim
R = 16            # rank
HPC = 6           # heads per core
NCORES = 8
SCALE = 1.0 / 8.0
ESC = 2.0 * SCALE  # exp scale
NCH = 6           # contraction chunks of E

LAST_RESULTS = None     # BassKernelResults of the most recent run (for test.py)


def _emit(tc):
    nc = tc.nc
    hTd = nc.dram_tensor("hTa", [E, S], BF16, kind="ExternalInput")
    wQd = nc.dram_tensor("weffQ", [E, 128], BF16, kind="ExternalInput")
    wKd = nc.dram_tensor("weffK", [E, 128], BF16, kind="ExternalInput")
    wQ2d = nc.dram_tensor("weffQ2", [E, 128], BF16, kind="ExternalInput")
    wK2d = nc.dram_tensor("weffK2", [E, 128], BF16, kind="ExternalInput")
    wvd = nc.dram_tensor("WvTa", [E, HPC * D], BF16, kind="ExternalInput")
    wod = nc.dram_tensor("WoT", [HPC * D, E], BF16, kind="ExternalInput")
    mkd = nc.dram_tensor("maskT48", [128, 48], F32, kind="ExternalInput")
    indKd = nc.dram_tensor("indK", [128, 6], BF16, kind="ExternalInput")
    bvecKd = nc.dram_tensor("bvecK", [128, 6], BF16, kind="ExternalInput")
    indMd = nc.dram_tensor("indM2", [128, 6], BF16, kind="ExternalInput")
    bvecMd = nc.dram_tensor("bvecM2", [128, 6], BF16, kind="ExternalInput")
    bvd = nc.dram_tensor("bv_bc", [128, HPC * D], F32, kind="ExternalInput")
    bod = nc.dram_tensor("bo2", [128, E], F32, kind="ExternalInput")
    idd = nc.dram_tensor("ident", [128, 128], BF16, kind="ExternalInput")
    outd = nc.dram_tensor("outp", [S, E], F32, kind="ExternalOutput")

    import contextlib
    stack = contextlib.ExitStack()
    const = stack.enter_context(tc.tile_pool(name="const", bufs=1))
    work = stack.enter_context(tc.tile_pool(name="work", bufs=4))
    ptp = stack.enter_context(tc.tile_pool(name="ptp", bufs=1))
    psp = stack.enter_context(tc.tile_pool(name="psp", bufs=2, space="PSUM"))

    def psA():
        return psp.tile([128, 1024], F32, name="psA", tag="psA", bufs=3)

    def psC():
        return psp.tile([128, 512], F32, name="psC", tag="psC", bufs=2)

    cp = nc.vector.tensor_copy

    # ---------------- DMA: weights first, two queues ----------------
    # sync queue: weff chunks, then small consts, wv, wo, bo
    wQ, wK, wQ2, wK2 = [], [], [], []
    for c in range(NCH):
        r0 = 128 * c
        wQ.append(const.tile([128, 128], BF16, name=f"wQ{c}", tag=f"wQ{c}"))
        nc.sync.dma_start(out=wQ[c][:, :], in_=wQd[r0:r0 + 128, :])
        wK.append(const.tile([128, 128], BF16, name=f"wK{c}", tag=f"wK{c}"))
        nc.sync.dma_start(out=wK[c][:, :], in_=wKd[r0:r0 + 128, :])
        wQ2.append(const.tile([128, 128], BF16, name=f"wQ2{c}", tag=f"wQ2{c}"))
        nc.sync.dma_start(out=wQ2[c][:, :], in_=wQ2d[r0:r0 + 128, :])
        wK2.append(const.tile([128, 128], BF16, name=f"wK2{c}", tag=f"wK2{c}"))
        nc.sync.dma_start(out=wK2[c][:, :], in_=wK2d[r0:r0 + 128, :])
    maskT = const.tile([128, 48], F32, name="maskT", tag="maskT")
    nc.sync.dma_start(out=maskT[:, :], in_=mkd[:, :])
    indK = const.tile([128, 6], BF16, name="indK", tag="indK")
    nc.sync.dma_start(out=indK[:, :], in_=indKd[:, :])
    bvecK = const.tile([128, 6], BF16, name="bvecK", tag="bvecK")
    nc.sync.dma_start(out=bvecK[:, :], in_=bvecKd[:, :])
    indM = const.tile([128, 6], BF16, name="indM", tag="indM")
    nc.sync.dma_start(out=indM[:, :], in_=indMd[:, :])
    bvecM = const.tile([128, 6], BF16, name="bvecM", tag="bvecM")
    nc.sync.dma_start(out=bvecM[:, :], in_=bvecMd[:, :])
    wv = []
    for c in range(NCH):
        wv.append(const.tile([128, HPC * D], BF16, name=f"wv{c}", tag=f"wv{c}"))
        nc.sync.dma_start(out=wv[c][:, :], in_=wvd[128 * c:128 * (c + 1), :])
    bv_bc = const.tile([128, HPC * D], F32, name="bv_bc", tag="bv_bc")
    nc.sync.dma_start(out=bv_bc[:, :], in_=bvd[:, :])
    ident = const.tile([128, 128], BF16, name="ident", tag="ident")
    nc.sync.dma_start(out=ident[:, :], in_=idd[:, :])
    wo = []
    for j in range(3):
        wo.append(const.tile([128, E], BF16, name=f"wo{j}", tag=f"wo{j}"))
        nc.sync.dma_start(out=wo[j][:, :], in_=wod[128 * j:128 * (j + 1), :])
    bo_bc = const.tile([128, E], F32, name="bo_bc", tag="bo_bc")
    nc.sync.dma_start(out=bo_bc[:, :], in_=bod[:, :])

    # gpsimd queue: hT chunks (critical path for qk projection)
    hT = []
    for c in range(NCH):
        hT.append(const.tile([128, S], BF16, name=f"hT{c}", tag=f"hT{c}"))
        nc.gpsimd.dma_start(out=hT[c][:, :], in_=hTd[128 * c:128 * (c + 1), :])

    # persistent SBUF tiles
    qaQ = const.tile([128, S], BF16, name="qaQ", tag="qaQ")    # q0 q1 q2
    qaQ2 = const.tile([128, S], BF16, name="qaQ2", tag="qaQ2")  # q3 q4 q5
    kaK = const.tile([128, S], BF16, name="kaK", tag="kaK")    # k0 k1 k2
    kaK2 = const.tile([128, S], BF16, name="kaK2", tag="kaK2")  # k3 k4 k5
    ksqK = const.tile([128, S], BF16, name="ksqK", tag="ksqK")
    ksqK2 = const.tile([128, S], BF16, name="ksqK2", tag="ksqK2")
    ckkT = const.tile([128, 48], F32, name="ckkT", tag="ckkT")
    vsb = [const.tile([128, HPC * (D + 1)], BF16, name=f"v{t}", tag=f"v{t}")
           for t in range(8)]
    ctxn = [const.tile([128, HPC * D], BF16, name=f"ctxn{s}", tag=f"ctxn{s}")
            for s in range(8)]
    ctxT = [const.tile([128, S], BF16, name=f"ctxT{j}", tag=f"ctxT{j}")
            for j in range(3)]
    # preset the ones columns of vsb (denominator trick)
    for t in range(8):
        vv = vsb[t][:, :].rearrange("p (h c) -> p h c", h=HPC)
        nc.vector.memset(vv[:, :, D:D + 1], 1.0)

    # ---------------- qk projection: 4 groups, chunk-streamed ----------------
    # Q/K/Q2 in full [128,1024] PSUM tiles; K2 in two [128,512] C slots.
    pcK, pcQ, pcQ2 = psA(), psA(), psA()
    pcK2 = [psC(), psC()]
    for c in range(NCH):
        st, sp = (c == 0), (c == NCH - 1)
        for n in range(2):
            sl = slice(512 * n, 512 * (n + 1))
            nc.tensor.matmul(out=pcK[:, sl], lhsT=wK[c][:, :],
                             rhs=hT[c][:, sl], start=st, stop=sp)
            nc.tensor.matmul(out=pcK2[n][:, 0:512], lhsT=wK2[c][:, :],
                             rhs=hT[c][:, sl], start=st, stop=sp)
            nc.tensor.matmul(out=pcQ[:, sl], lhsT=wQ[c][:, :],
                             rhs=hT[c][:, sl], start=st, stop=sp)
            nc.tensor.matmul(out=pcQ2[:, sl], lhsT=wQ2[c][:, :],
                             rhs=hT[c][:, sl], start=st, stop=sp)

    # evacuations: ACT takes K + K2 (critical for kk), DVE takes Q, Q2
    nc.scalar.activation(out=kaK[:, :], in_=pcK[:, :], func=AF.Copy)
    nc.scalar.activation(out=kaK2[:, 0:512], in_=pcK2[0][:, 0:512], func=AF.Copy)
    nc.scalar.activation(out=kaK2[:, 512:1024], in_=pcK2[1][:, 0:512],
                         func=AF.Copy)
    cp(qaQ[:, :], pcQ[:, :])
    cp(qaQ2[:, :], pcQ2[:, :])

    # ksq + kk (all 6 heads, one pass)
    nc.vector.tensor_mul(ksqK[:, :], kaK[:, :], kaK[:, :])
    nc.vector.tensor_mul(ksqK2[:, :], kaK2[:, :], kaK2[:, :])
    pkk = psC()
    for t in range(8):
        tsl = slice(128 * t, 128 * (t + 1))
        nc.tensor.matmul(out=pkk[:, 6 * t:6 * t + 6], lhsT=ksqK[:, tsl],
                         rhs=indK[:, :], start=True, stop=False)
        nc.tensor.matmul(out=pkk[:, 6 * t:6 * t + 6], lhsT=kaK[:, tsl],
                         rhs=bvecK[:, :], start=False, stop=False)
        nc.tensor.matmul(out=pkk[:, 6 * t:6 * t + 6], lhsT=ksqK2[:, tsl],
                         rhs=indM[:, :], start=False, stop=False)
        nc.tensor.matmul(out=pkk[:, 6 * t:6 * t + 6], lhsT=kaK2[:, tsl],
                         rhs=bvecM[:, :], start=False, stop=True)
    nc.vector.scalar_tensor_tensor(out=ckkT[:, :], in0=pkk[:, 0:48], scalar=0.0,
                                   in1=maskT[:, :], op0=ALU.bypass, op1=ALU.add)

    def qa_ap(h):
        return qaQ if h < 3 else qaQ2

    def ka_ap(h):
        return kaK if h < 3 else kaK2

    def base(h):
        return 32 * h if h < 3 else 32 * (h - 3)

    def ckk_col(h, t):
        return ckkT[:, 6 * t + h:6 * t + h + 1]

    # ---------------- per-head pipeline ----------------
    # fillers: units of PE work executed between score matmuls
    fillers = []

    def run_filler():
        if fillers:
            fillers.pop(0)()

    def v_tile(s):
        def f():
            pv = psC()
            for c in range(NCH):
                nc.tensor.matmul(out=pv[:, 0:HPC * D],
                                 lhsT=hT[c][:, 128 * s:128 * (s + 1)],
                                 rhs=wv[c][:, :],
                                 start=(c == 0), stop=(c == NCH - 1))
            vv = vsb[s][:, :].rearrange("p (h c) -> p h c", h=HPC)
            nc.vector.scalar_tensor_tensor(
                out=vv[:, :, 0:D],
                in0=pv[:, 0:HPC * D].rearrange("p (h c) -> p h c", h=HPC),
                scalar=0.0,
                in1=bv_bc[:, :].rearrange("p (h c) -> p h c", h=HPC),
                op0=ALU.bypass, op1=ALU.add)
        return f

    def ctx_quad(h, q):
        def f():
            px = psC()
            for i in range(4):
                s = 4 * q + i
                for t in range(8):
                    nc.tensor.matmul(
                        out=px[:, 65 * i:65 * i + 65],
                        lhsT=ptiles[h % 2][t][:, 128 * s:128 * (s + 1)],
                        rhs=vsb[t][:, (D + 1) * h:(D + 1) * (h + 1)],
                        start=(t == 0), stop=(t == 7))
            rec4 = work.tile([128, 4], F32, name="rec4", tag="rec4")
            pxv = px[:, 0:260].rearrange("p (i c) -> p i c", i=4)
            den = pxv[:, :, D:D + 1].rearrange("p i c -> p (i c)")
            nc.vector.reciprocal(rec4[:, :], den)
            for i in range(4):
                s = 4 * q + i
                nc.vector.tensor_scalar_mul(ctxn[s][:, D * h:D * (h + 1)],
                                            px[:, 65 * i:65 * i + D],
                                            rec4[:, i:i + 1])
        return f

    def transp(j, s):
        def f():
            ct = psC()
            pt = ct[:, :].bitcast(BF16)[:, 0:128]
            nc.tensor.transpose(pt, ctxn[s][:, 128 * j:128 * (j + 1)],
                                ident[:, :])
            cp(ctxT[j][:, 128 * s:128 * (s + 1)], pt)
        return f

    # pts double-buffered across heads
    ptiles = [[ptp.tile([128, S], BF16, name=f"pt{p}_{t}", tag=f"pt{p}_{t}")
               for t in range(8)] for p in range(2)]

    def scores_head(h):
        qa, ka, b = qa_ap(h), ka_ap(h), base(h)
        for t in range(8):
            pc = psA()
            for n in range(2):
                nc.tensor.matmul(
                    out=pc[:, 512 * n:512 * (n + 1)],
                    lhsT=ka[b:b + R, 128 * t:128 * (t + 1)],
                    rhs=qa[b:b + R, 512 * n:512 * (n + 1)],
                    start=True, stop=True)
            nc.scalar.activation(out=ptiles[h % 2][t][:, :], in_=pc[:, :],
                                 func=AF.Exp, bias=ckk_col(h, t), scale=ESC)
            run_filler()
            run_filler()

    # head 0: kk_part2 + v tiles fill the gaps
    fillers = [v_tile(s) for s in range(8)]
    scores_head(0)
    for h in range(1, HPC):
        # while exp of head h streams, compute ctx of head h-1 (+ transposes)
        fillers = [ctx_quad(h - 1, 0), ctx_quad(h - 1, 1)]
        if h >= 2 and h % 2 == 0:
            j = h // 2 - 1
            fillers += [transp(j, s) for s in range(8)]
        scores_head(h)
        while fillers:
            run_filler()
    fillers = [ctx_quad(HPC - 1, 0), ctx_quad(HPC - 1, 1)]
    fillers += [transp(2, s) for s in range(8)]
    while fillers:
        run_filler()

    # ---------------- out projection, 2-wave tail ----------------
    for w in range(4):
        pos = [psA() for _ in range(2)]
        for i in range(2):
            s = 2 * w + i
            for n0, nw in ((0, 512), (512, 256)):
                for j in range(3):
                    nc.tensor.matmul(out=pos[i][:, n0:n0 + nw],
                                     lhsT=ctxT[j][:, 128 * s:128 * (s + 1)],
                                     rhs=wo[j][:, n0:n0 + nw],
                                     start=(j == 0), stop=(j == 2))
        for i in range(2):
            s = 2 * w + i
            osb = work.tile([128, E], F32, name="osb", tag="osb", bufs=3)
            nc.vector.scalar_tensor_tensor(
                out=osb[:, :], in0=pos[i][:, 0:E], scalar=0.0,
                in1=bo_bc[:, :], op0=ALU.bypass, op1=ALU.add)
            nc.sync.dma_start(out=outd[128 * s:128 * (s + 1), :], in_=osb[:, :])

    stack.close()


_NC_CACHE = None


def _build():
    global _NC_CACHE
    if _NC_CACHE is None:
        nc = bacc.Bacc("TRN2", target_bir_lowering=False, debug=False,
                       enable_asserts=True, num_devices=NCORES)
        with tile.TileContext(nc) as tc:
            _emit(tc)
        nc.compile()
        _NC_CACHE = nc
    return _NC_CACHE


def kernel(hidden_states, attention_mask, Wq, bq, Wk, bk, Wv, bv, Wo, bo, A,
           **_ignored):
    global LAST_RESULTS
    hidden_states = np.asarray(hidden_states, np.float32)
    attention_mask = np.asarray(attention_mask, np.float32)
    Wq, bq = np.asarray(Wq, np.float32), np.asarray(bq, np.float32)
    Wk, bk = np.asarray(Wk, np.float32), np.asarray(bk, np.float32)
    Wv, bv = np.asarray(Wv, np.float32), np.asarray(bv, np.float32)
    Wo, bo = np.asarray(Wo, np.float32), np.asarray(bo, np.float32)
    A = np.asarray(A, np.float32)

    B = hidden_states.shape[0]
    nc = _build()

    bf = ml_dtypes.bfloat16
    ident = np.eye(128, dtype=np.float32)
    in_maps = []
    for c in range(NCORES):
        b = c // 2
        h0 = HPC * (c % 2)
        sl = slice(h0 * D, (h0 + HPC) * D)

        # host-folded Weff = A^T W (per head), 32-col spacing, pads zero
        weffQ = np.zeros((E, 128), np.float32)
        weffK = np.zeros((E, 128), np.float32)
        weffQ2 = np.zeros((E, 128), np.float32)
        weffK2 = np.zeros((E, 128), np.float32)
        bvecK = np.zeros((128, 6), np.float32)
        bvecM2 = np.zeros((128, 6), np.float32)
        indKm = np.zeros((128, 6), np.float32)
        indM2 = np.zeros((128, 6), np.float32)
        for h in range(HPC):
            Ah = A[h0 + h]                                  # (64, 16)
            hd = slice((h0 + h) * D, (h0 + h + 1) * D)
            AtWq = Ah.T @ Wq[hd]                            # (16, 768)
            AtWk = Ah.T @ Wk[hd]
            bqA = Ah.T @ bq[hd]                             # (16,)
            bkA = Ah.T @ bk[hd]
            bvec = ESC * (bqA - bkA)
            if h < 3:
                weffQ[:, 32 * h:32 * h + R] = AtWq.T
                weffK[:, 32 * h:32 * h + R] = AtWk.T
                bvecK[32 * h:32 * h + R, h] = bvec
                indKm[32 * h:32 * h + 32, h] = -SCALE
            else:
                g = h - 3
                weffQ2[:, 32 * g:32 * g + R] = AtWq.T
                weffK2[:, 32 * g:32 * g + R] = AtWk.T
                bvecM2[32 * g:32 * g + R, h] = bvec
                indM2[32 * g:32 * g + 32, h] = -SCALE

        maskT48 = np.repeat(
            attention_mask[b, 0, 0].reshape(8, 128).T, 6, axis=1)  # (128, 48)

        in_maps.append({
            "hTa": np.ascontiguousarray(hidden_states[b].T.astype(bf)),
            "weffQ": np.ascontiguousarray(weffQ.astype(bf)),
            "weffK": np.ascontiguousarray(weffK.astype(bf)),
            "weffQ2": np.ascontiguousarray(weffQ2.astype(bf)),
            "weffK2": np.ascontiguousarray(weffK2.astype(bf)),
            "WvTa": np.ascontiguousarray(Wv[sl].T.astype(bf)),
            "WoT": np.ascontiguousarray(Wo[:, sl].T.astype(bf)),
            "maskT48": np.ascontiguousarray(maskT48.astype(np.float32)),
            "indK": np.ascontiguousarray(indKm.astype(bf)),
            "bvecK": np.ascontiguousarray(bvecK.astype(bf)),
            "indM2": np.ascontiguousarray(indM2.astype(bf)),
            "bvecM2": np.ascontiguousarray(bvecM2.astype(bf)),
            "bv_bc": np.ascontiguousarray(
                np.broadcast_to(bv[sl], (128, HPC * D)).astype(np.float32)),
            "bo2": np.ascontiguousarray(
                np.broadcast_to(bo / 2.0, (128, E)).astype(np.float32)),
            "ident": np.ascontiguousarray(ident.astype(bf)),
        })

    res = run_bass_kernel_spmd(nc, in_maps, list(range(NCORES)),
                               trace=bool(os.environ.get("KERNEL_TRACE")))
    LAST_RESULTS = res
    parts = [res.results[c]["outp"] for c in range(NCORES)]
    out = np.stack([parts[2 * b] + parts[2 * b + 1] for b in range(B)], 0)
    return np.ascontiguousarray(out.astype(np.float32))


# revision 11
# speedup vs baseline: 1.1410x; 1.0485x over previous
"""Trainium2 Bass kernel for nn_CurvedMultiHeadAttention (B=4, S=1024, E=768, H=12, D=64, R=16).

Sharding: 8 cores; core c handles batch b=c//2 and heads h0=6*(c%2) .. h0+5.
Each core computes its 6 heads' out-projection contribution (bo/2 folded in);
the host sums the two partials per batch element.

Math (validated ~1e-6 vs reference in fp64):
 - softmax over keys is invariant to per-query shifts => qq term drops.
 - EPS*I part of G_h contributes <1e-5 to scores => dropped.
 - Weff_q = A_h^T Wq_h (16x768 per head) is folded on the HOST (weight-only
   transform), so the device projects hidden straight to qA/kA (16 dims/head,
   padded to 32 for PE quadrant alignment).
 - q/k biases fold into the per-key exp bias:
     exp arg = ESC*(kA0.qA0) + ckk[t],
     ckk = -SCALE*|kA0|^2 + kA0.(ESC*(bqA-bkA)) + mask
   computed with two small indicator matmuls (ksq x ind + kA x bvec).
 - softmax denominator comes free as a ones column appended to v in the ctx
   matmul; normalization happens during PSUM evacuation (per-partition scalar).

Schedule: weights-first multi-queue DMA; qk projection (PE) streams behind the
hT chunk DMAs; exp (ScalarE) starts ~16us in and runs back-to-back; v
projection, ctx, transposes and out-projection j-chunks fill PE slack under
the exp stream; 2-wave out-projection tail.
"""

import os
import numpy as np
import ml_dtypes

import concourse.bass as bass
import concourse.tile as tile
from concourse import bacc
from concourse import mybir
from concourse.bass_utils import run_bass_kernel_spmd

F32 = mybir.dt.float32
BF16 = mybir.dt.bfloat16
AF = mybir.ActivationFunctionType
ALU = mybir.AluOpType

S = 1024          # sequence length
E = 768           # embed
D = 64            # head dim
R = 16            # rank
HPC = 6           # heads per core
NCORES = 8
SCALE = 1.0 / 8.0
ESC = 2.0 * SCALE  # exp scale
NCH = 6           # contraction chunks of E

LAST_RESULTS = None     # BassKernelResults of the most recent run (for test.py)


def _emit(tc):
    nc = tc.nc
    hTd = nc.dram_tensor("hTa", [E, S], BF16, kind="ExternalInput")
    wQd = nc.dram_tensor("weffQ", [128, NCH * 128], BF16, kind="ExternalInput")
    wKd = nc.dram_tensor("weffK", [128, NCH * 128], BF16, kind="ExternalInput")
    wQ2d = nc.dram_tensor("weffQ2", [128, NCH * 128], BF16, kind="ExternalInput")
    wK2d = nc.dram_tensor("weffK2", [128, NCH * 128], BF16, kind="ExternalInput")
    wvd = nc.dram_tensor("WvTa", [E, HPC * D], BF16, kind="ExternalInput")
    wod = nc.dram_tensor("WoT", [HPC * D, E], BF16, kind="ExternalInput")
    mkd = nc.dram_tensor("maskT48", [128, 48], F32, kind="ExternalInput")
    indKd = nc.dram_tensor("indK", [128, 6], BF16, kind="ExternalInput")
    bvecKd = nc.dram_tensor("bvecK", [128, 6], BF16, kind="ExternalInput")
    indMd = nc.dram_tensor("indM2", [128, 6], BF16, kind="ExternalInput")
    bvecMd = nc.dram_tensor("bvecM2", [128, 6], BF16, kind="ExternalInput")
    bvd = nc.dram_tensor("bv_bc", [128, HPC * D], F32, kind="ExternalInput")
    bod = nc.dram_tensor("bo2", [128, E], F32, kind="ExternalInput")
    idd = nc.dram_tensor("ident", [128, 128], BF16, kind="ExternalInput")
    outd = nc.dram_tensor("outp", [S, E], F32, kind="ExternalOutput")

    import contextlib
    stack = contextlib.ExitStack()
    const = stack.enter_context(tc.tile_pool(name="const", bufs=1))
    work = stack.enter_context(tc.tile_pool(name="work", bufs=4))
    ptp = stack.enter_context(tc.tile_pool(name="ptp", bufs=1))
    psp = stack.enter_context(tc.tile_pool(name="psp", bufs=2, space="PSUM"))

    def psA():
        return psp.tile([128, 1024], F32, name="psA", tag="psA", bufs=3)

    def psC():
        return psp.tile([128, 512], F32, name="psC", tag="psC", bufs=2)

    cp = nc.vector.tensor_copy

    # ---------------- DMA: weights first, two queues ----------------
    # sync queue: weff chunks, then small consts, wv, wo, bo
    wKt = const.tile([128, NCH * 128], BF16, name="wKt", tag="wKt")
    nc.sync.dma_start(out=wKt[:, :], in_=wKd[:, :])
    wK2t = const.tile([128, NCH * 128], BF16, name="wK2t", tag="wK2t")
    nc.sync.dma_start(out=wK2t[:, :], in_=wK2d[:, :])
    wQt = const.tile([128, NCH * 128], BF16, name="wQt", tag="wQt")
    nc.sync.dma_start(out=wQt[:, :], in_=wQd[:, :])
    wQ2t = const.tile([128, NCH * 128], BF16, name="wQ2t", tag="wQ2t")
    nc.sync.dma_start(out=wQ2t[:, :], in_=wQ2d[:, :])
    wQ = [wQt[:, 128 * c:128 * (c + 1)] for c in range(NCH)]
    wK = [wKt[:, 128 * c:128 * (c + 1)] for c in range(NCH)]
    wQ2 = [wQ2t[:, 128 * c:128 * (c + 1)] for c in range(NCH)]
    wK2 = [wK2t[:, 128 * c:128 * (c + 1)] for c in range(NCH)]
    maskT = const.tile([128, 48], F32, name="maskT", tag="maskT")
    nc.sync.dma_start(out=maskT[:, :], in_=mkd[:, :])
    indK = const.tile([128, 6], BF16, name="indK", tag="indK")
    nc.sync.dma_start(out=indK[:, :], in_=indKd[:, :])
    bvecK = const.tile([128, 6], BF16, name="bvecK", tag="bvecK")
    nc.sync.dma_start(out=bvecK[:, :], in_=bvecKd[:, :])
    indM = const.tile([128, 6], BF16, name="indM", tag="indM")
    nc.sync.dma_start(out=indM[:, :], in_=indMd[:, :])
    bvecM = const.tile([128, 6], BF16, name="bvecM", tag="bvecM")
    nc.sync.dma_start(out=bvecM[:, :], in_=bvecMd[:, :])
    wv = []
    for c in range(NCH):
        wv.append(const.tile([128, HPC * D], BF16, name=f"wv{c}", tag=f"wv{c}"))
        nc.sync.dma_start(out=wv[c][:, :], in_=wvd[128 * c:128 * (c + 1), :])
    bv_bc = const.tile([128, HPC * D], F32, name="bv_bc", tag="bv_bc")
    nc.sync.dma_start(out=bv_bc[:, :], in_=bvd[:, :])
    ident = const.tile([128, 128], BF16, name="ident", tag="ident")
    nc.sync.dma_start(out=ident[:, :], in_=idd[:, :])
    wo = []
    for j in range(3):
        wo.append(const.tile([128, E], BF16, name=f"wo{j}", tag=f"wo{j}"))
        nc.sync.dma_start(out=wo[j][:, :], in_=wod[128 * j:128 * (j + 1), :])
    bo_bc = const.tile([128, E], F32, name="bo_bc", tag="bo_bc")
    nc.sync.dma_start(out=bo_bc[:, :], in_=bod[:, :])

    # gpsimd queue: hT chunks (critical path for qk projection)
    hT = []
    for c in range(NCH):
        hT.append(const.tile([128, S], BF16, name=f"hT{c}", tag=f"hT{c}"))
        nc.gpsimd.dma_start(out=hT[c][:, :], in_=hTd[128 * c:128 * (c + 1), :])

    # persistent SBUF tiles
    qaQ = const.tile([128, S], BF16, name="qaQ", tag="qaQ")    # q0 q1 q2
    qaQ2 = const.tile([128, S], BF16, name="qaQ2", tag="qaQ2")  # q3 q4 q5
    kaK = const.tile([128, S], BF16, name="kaK", tag="kaK")    # k0 k1 k2
    kaK2 = const.tile([128, S], BF16, name="kaK2", tag="kaK2")  # k3 k4 k5
    ksqK = const.tile([128, S], BF16, name="ksqK", tag="ksqK")
    ksqK2 = const.tile([128, S], BF16, name="ksqK2", tag="ksqK2")
    ckkT = const.tile([128, 48], F32, name="ckkT", tag="ckkT")
    vsb = [const.tile([128, HPC * (D + 1)], BF16, name=f"v{t}", tag=f"v{t}")
           for t in range(8)]
    ctxn = [const.tile([128, HPC * D], BF16, name=f"ctxn{s}", tag=f"ctxn{s}")
            for s in range(8)]
    ctxT = [const.tile([128, S], BF16, name=f"ctxT{j}", tag=f"ctxT{j}")
            for j in range(3)]
    # preset the ones columns of vsb (denominator trick)
    for t in range(8):
        vv = vsb[t][:, :].rearrange("p (h c) -> p h c", h=HPC)
        nc.vector.memset(vv[:, :, D:D + 1], 1.0)

    # ---------------- qk projection: 4 groups, chunk-streamed ----------------
    # Q/K/Q2 in full [128,1024] PSUM tiles; K2 in two [128,512] C slots.
    pcK, pcQ, pcQ2 = psA(), psA(), psA()
    pcK2 = [psC(), psC()]
    for c in range(NCH):
        st, sp = (c == 0), (c == NCH - 1)
        for n in range(2):
            sl = slice(512 * n, 512 * (n + 1))
            nc.tensor.matmul(out=pcK[:, sl], lhsT=wK[c],
                             rhs=hT[c][:, sl], start=st, stop=sp)
            nc.tensor.matmul(out=pcK2[n][:, 0:512], lhsT=wK2[c],
                             rhs=hT[c][:, sl], start=st, stop=sp)
            nc.tensor.matmul(out=pcQ[:, sl], lhsT=wQ[c],
                             rhs=hT[c][:, sl], start=st, stop=sp)
            nc.tensor.matmul(out=pcQ2[:, sl], lhsT=wQ2[c],
                             rhs=hT[c][:, sl], start=st, stop=sp)

    # evacuations: ACT takes K + K2 (critical for kk), DVE takes Q, Q2
    nc.scalar.activation(out=kaK[:, :], in_=pcK[:, :], func=AF.Copy)
    nc.scalar.activation(out=kaK2[:, 0:512], in_=pcK2[0][:, 0:512], func=AF.Copy)
    nc.scalar.activation(out=kaK2[:, 512:1024], in_=pcK2[1][:, 0:512],
                         func=AF.Copy)
    cp(qaQ[:, :], pcQ[:, :])
    cp(qaQ2[:, :], pcQ2[:, :])

    # ksq + kk (all 6 heads, one pass)
    nc.vector.tensor_mul(ksqK[:, :], kaK[:, :], kaK[:, :])
    nc.vector.tensor_mul(ksqK2[:, :], kaK2[:, :], kaK2[:, :])
    pkk = psC()
    for t in range(8):
        tsl = slice(128 * t, 128 * (t + 1))
        nc.tensor.matmul(out=pkk[:, 6 * t:6 * t + 6], lhsT=ksqK[:, tsl],
                         rhs=indK[:, :], start=True, stop=False)
        nc.tensor.matmul(out=pkk[:, 6 * t:6 * t + 6], lhsT=kaK[:, tsl],
                         rhs=bvecK[:, :], start=False, stop=False)
        nc.tensor.matmul(out=pkk[:, 6 * t:6 * t + 6], lhsT=ksqK2[:, tsl],
                         rhs=indM[:, :], start=False, stop=False)
        nc.tensor.matmul(out=pkk[:, 6 * t:6 * t + 6], lhsT=kaK2[:, tsl],
                         rhs=bvecM[:, :], start=False, stop=True)
    nc.vector.scalar_tensor_tensor(out=ckkT[:, :], in0=pkk[:, 0:48], scalar=0.0,
                                   in1=maskT[:, :], op0=ALU.bypass, op1=ALU.add)

    def qa_ap(h):
        return qaQ if h < 3 else qaQ2

    def ka_ap(h):
        return kaK if h < 3 else kaK2

    def base(h):
        return 32 * h if h < 3 else 32 * (h - 3)

    def ckk_col(h, t):
        return ckkT[:, 6 * t + h:6 * t + h + 1]

    # ---------------- per-head pipeline ----------------
    # fillers: units of PE work executed between score matmuls
    fillers = []

    def run_filler():
        if fillers:
            fillers.pop(0)()

    def v_tile(s):
        def f():
            pv = psC()
            for c in range(NCH):
                nc.tensor.matmul(out=pv[:, 0:HPC * D],
                                 lhsT=hT[c][:, 128 * s:128 * (s + 1)],
                                 rhs=wv[c][:, :],
                                 start=(c == 0), stop=(c == NCH - 1))
            vv = vsb[s][:, :].rearrange("p (h c) -> p h c", h=HPC)
            nc.vector.scalar_tensor_tensor(
                out=vv[:, :, 0:D],
                in0=pv[:, 0:HPC * D].rearrange("p (h c) -> p h c", h=HPC),
                scalar=0.0,
                in1=bv_bc[:, :].rearrange("p (h c) -> p h c", h=HPC),
                op0=ALU.bypass, op1=ALU.add)
        return f

    def ctx_quad(h, q):
        def f():
            px = psC()
            for i in range(4):
                s = 4 * q + i
                for t in range(8):
                    nc.tensor.matmul(
                        out=px[:, 65 * i:65 * i + 65],
                        lhsT=ptiles[h % 2][t][:, 128 * s:128 * (s + 1)],
                        rhs=vsb[t][:, (D + 1) * h:(D + 1) * (h + 1)],
                        start=(t == 0), stop=(t == 7))
            rec4 = work.tile([128, 4], F32, name="rec4", tag="rec4")
            pxv = px[:, 0:260].rearrange("p (i c) -> p i c", i=4)
            den = pxv[:, :, D:D + 1].rearrange("p i c -> p (i c)")
            nc.vector.reciprocal(rec4[:, :], den)
            for i in range(4):
                s = 4 * q + i
                nc.vector.tensor_scalar_mul(ctxn[s][:, D * h:D * (h + 1)],
                                            px[:, 65 * i:65 * i + D],
                                            rec4[:, i:i + 1])
        return f

    def transp(j, s):
        def f():
            ct = psC()
            pt = ct[:, :].bitcast(BF16)[:, 0:128]
            nc.tensor.transpose(pt, ctxn[s][:, 128 * j:128 * (j + 1)],
                                ident[:, :])
            cp(ctxT[j][:, 128 * s:128 * (s + 1)], pt)
        return f

    # pts double-buffered across heads
    ptiles = [[ptp.tile([128, S], BF16, name=f"pt{p}_{t}", tag=f"pt{p}_{t}")
               for t in range(8)] for p in range(2)]

    def scores_head(h):
        qa, ka, b = qa_ap(h), ka_ap(h), base(h)
        for t in range(8):
            pc = psA()
            for n in range(2):
                nc.tensor.matmul(
                    out=pc[:, 512 * n:512 * (n + 1)],
                    lhsT=ka[b:b + R, 128 * t:128 * (t + 1)],
                    rhs=qa[b:b + R, 512 * n:512 * (n + 1)],
                    start=True, stop=True)
            nc.scalar.activation(out=ptiles[h % 2][t][:, :], in_=pc[:, :],
                                 func=AF.Exp, bias=ckk_col(h, t), scale=ESC)
            run_filler()
            run_filler()

    # head 0: kk_part2 + v tiles fill the gaps
    fillers = [v_tile(s) for s in range(8)]
    scores_head(0)
    for h in range(1, HPC):
        # while exp of head h streams, compute ctx of head h-1 (+ transposes)
        fillers = [ctx_quad(h - 1, 0), ctx_quad(h - 1, 1)]
        if h >= 2 and h % 2 == 0:
            j = h // 2 - 1
            fillers += [transp(j, s) for s in range(8)]
        scores_head(h)
        while fillers:
            run_filler()
    fillers = [ctx_quad(HPC - 1, 0), ctx_quad(HPC - 1, 1)]
    fillers += [transp(2, s) for s in range(8)]
    while fillers:
        run_filler()

    # ---------------- out projection, 2-wave tail ----------------
    for w in range(4):
        pos = [psA() for _ in range(2)]
        for i in range(2):
            s = 2 * w + i
            for n0, nw in ((0, 512), (512, 256)):
                for j in range(3):
                    nc.tensor.matmul(out=pos[i][:, n0:n0 + nw],
                                     lhsT=ctxT[j][:, 128 * s:128 * (s + 1)],
                                     rhs=wo[j][:, n0:n0 + nw],
                                     start=(j == 0), stop=(j == 2))
        for i in range(2):
            s = 2 * w + i
            osb = work.tile([128, E], F32, name="osb", tag="osb", bufs=3)
            nc.vector.scalar_tensor_tensor(
                out=osb[:, :], in0=pos[i][:, 0:E], scalar=0.0,
                in1=bo_bc[:, :], op0=ALU.bypass, op1=ALU.add)
            nc.sync.dma_start(out=outd[128 * s:128 * (s + 1), :], in_=osb[:, :])

    stack.close()


_NC_CACHE = None


def _build():
    global _NC_CACHE
    if _NC_CACHE is None:
        nc = bacc.Bacc("TRN2", target_bir_lowering=False, debug=False,
                       enable_asserts=True, num_devices=NCORES)
        with tile.TileContext(nc) as tc:
            _emit(tc)
        nc.compile()
        _NC_CACHE = nc
    return _NC_CACHE


def kernel(hidden_states, attention_mask, Wq, bq, Wk, bk, Wv, bv, Wo, bo, A,
           **_ignored):
    global LAST_RESULTS
    hidden_states = np.asarray(hidden_states, np.float32)
    attention_mask = np.asarray(attention_mask, np.float32)
    Wq, bq = np.asarray(Wq, np.float32), np.asarray(bq, np.float32)
    Wk, bk = np.asarray(Wk, np.float32), np.asarray(bk, np.float32)
    Wv, bv = np.asarray(Wv, np.float32), np.asarray(bv, np.float32)
    Wo, bo = np.asarray(Wo, np.float32), np.asarray(bo, np.float32)
    A = np.asarray(A, np.float32)

    B = hidden_states.shape[0]
    nc = _build()

    bf = ml_dtypes.bfloat16
    ident = np.eye(128, dtype=np.float32)
    in_maps = []
    for c in range(NCORES):
        b = c // 2
        h0 = HPC * (c % 2)
        sl = slice(h0 * D, (h0 + HPC) * D)

        # host-folded Weff = A^T W (per head), 32-col spacing, pads zero
        weffQ = np.zeros((E, 128), np.float32)
        weffK = np.zeros((E, 128), np.float32)
        weffQ2 = np.zeros((E, 128), np.float32)
        weffK2 = np.zeros((E, 128), np.float32)
        bvecK = np.zeros((128, 6), np.float32)
        bvecM2 = np.zeros((128, 6), np.float32)
        indKm = np.zeros((128, 6), np.float32)
        indM2 = np.zeros((128, 6), np.float32)
        for h in range(HPC):
            Ah = A[h0 + h]                                  # (64, 16)
            hd = slice((h0 + h) * D, (h0 + h + 1) * D)
            AtWq = Ah.T @ Wq[hd]                            # (16, 768)
            AtWk = Ah.T @ Wk[hd]
            bqA = Ah.T @ bq[hd]                             # (16,)
            bkA = Ah.T @ bk[hd]
            bvec = ESC * (bqA - bkA)
            if h < 3:
                weffQ[:, 32 * h:32 * h + R] = AtWq.T
                weffK[:, 32 * h:32 * h + R] = AtWk.T
                bvecK[32 * h:32 * h + R, h] = bvec
                indKm[32 * h:32 * h + 32, h] = -SCALE
            else:
                g = h - 3
                weffQ2[:, 32 * g:32 * g + R] = AtWq.T
                weffK2[:, 32 * g:32 * g + R] = AtWk.T
                bvecM2[32 * g:32 * g + R, h] = bvec
                indM2[32 * g:32 * g + 32, h] = -SCALE

        maskT48 = np.repeat(
            attention_mask[b, 0, 0].reshape(8, 128).T, 6, axis=1)  # (128, 48)

        def wpack(w):
            # (768, 128) -> (128, 6*128): out[p, 128c+m] = w[128c+p, m]
            return np.ascontiguousarray(
                w.reshape(NCH, 128, 128).transpose(1, 0, 2).reshape(128, -1)
                .astype(bf))

        in_maps.append({
            "hTa": np.ascontiguousarray(hidden_states[b].T.astype(bf)),
            "weffQ": wpack(weffQ),
            "weffK": wpack(weffK),
            "weffQ2": wpack(weffQ2),
            "weffK2": wpack(weffK2),
            "WvTa": np.ascontiguousarray(Wv[sl].T.astype(bf)),
            "WoT": np.ascontiguousarray(Wo[:, sl].T.astype(bf)),
            "maskT48": np.ascontiguousarray(maskT48.astype(np.float32)),
            "indK": np.ascontiguousarray(indKm.astype(bf)),
            "bvecK": np.ascontiguousarray(bvecK.astype(bf)),
            "indM2": np.ascontiguousarray(indM2.astype(bf)),
            "bvecM2": np.ascontiguousarray(bvecM2.astype(bf)),
            "bv_bc": np.ascontiguousarray(
                np.broadcast_to(bv[sl], (128, HPC * D)).astype(np.float32)),
            "bo2": np.ascontiguousarray(
                np.broadcast_to(bo / 2.0, (128, E)).astype(np.float32)),
            "ident": np.ascontiguousarray(ident.astype(bf)),
        })

    res = run_bass_kernel_spmd(nc, in_maps, list(range(NCORES)),
                               trace=bool(os.environ.get("KERNEL_TRACE")))
    LAST_RESULTS = res
    parts = [res.results[c]["outp"] for c in range(NCORES)]
    out = np.stack([parts[2 * b] + parts[2 * b + 1] for b in range(B)], 0)
    return np.ascontiguousarray(out.astype(np.float32))
